# revision 1
# baseline (speedup 1.0000x reference)
"""DCRNN Trainium2 kernel: 8-way node sharding with on-device A^2 operators.

Decomposition (validated in mirror.py):
- A row-normalized on host; per-core operator column-slices R_A = A^T[:, sh],
  R_AT = A[:, sh] fed as inputs. On device, R_A2 = (A^T)^2[:, sh] is computed
  by streaming A once; R_AT2 = A^2[:, sh] is derived from R_A2 via an
  AllToAll block exchange + PE transposes.
- Activations live feature-major per shard: state tiles [H=64, (b, n)=512].
- Each diffused tensor gets a "bundle" [64, (b, op5, n256)] = identity + the
  4 operator applications, produced by f32r matmuls whose lhsT is the
  AllGathered node-major activation [2048, cols] streamed in 128-row chunks
  against the resident operator pair tiles (rhs [128, 512], full f32r rate).
- Projections contract (op, feat) with K=64 W slices against bundle slices;
  gates/cand ACT and the GRU update are row-local DVE/ACT work.
- 10 AllGathers total; xp diffusions are batched up-front (xp is global).

Hardware constraints honored (probed on trn2):
- every instruction <= 1 sync wait -> must build on bacc.Bacc + nc.compile()
  (generate_event_semaphores legalizes)
- f32r matmul inputs must be produced f32r (DMA-bitcast or DVE-copy out)
- 2-input DVE ops and matmul lhsT/rhs need equal base partitions
- DMA cannot read PSUM; transposes bounce PSUM -> DVE copy -> SBUF
"""
import numpy as np
import concourse.bass as bass
import concourse.bacc as bacc
import concourse.tile as tile
from concourse import mybir
from concourse.bass_utils import run_bass_kernel_spmd

F32 = mybir.dt.float32
F32R = mybir.dt.float32r
BF16 = mybir.dt.bfloat16
AF = mybir.ActivationFunctionType

N, H, B, SEQ, L = 2048, 64, 2, 3, 2
W = 8            # cores
NS = N // W      # 256 nodes per shard
KT = N // 128    # 16 contraction tiles
BN = B * NS      # 512 = (b, n) free size
RG = [list(range(W))]
PHASES = [("enc", 0), ("enc", 1), ("dec", 0), ("dec", 1)]  # dram row order
HB_BUFS, RHB_BUFS = 4, 1


def build_program():
    nc = bacc.Bacc(None, num_devices=W, name="dcrnn")

    # ---- DRAM inputs (per core) ----
    r_a = nc.dram_tensor("r_a", [N, NS], F32, kind="ExternalInput")
    r_at = nc.dram_tensor("r_at", [N, NS], F32, kind="ExternalInput")
    a_full = nc.dram_tensor("a_full", [N, N], F32, kind="ExternalInput")
    xp_nm = nc.dram_tensor("xp_nm", [N, SEQ * 128], F32, kind="ExternalInput")
    xp_fm = nc.dram_tensor("xp_fm", [H, SEQ * BN], F32, kind="ExternalInput")
    wg_in = nc.dram_tensor("wg_in", [4, 5 * 128, 2 * H], F32, kind="ExternalInput")
    wc_in = nc.dram_tensor("wc_in", [4, 5 * 128, H], F32, kind="ExternalInput")
    bg_in = nc.dram_tensor("bg_in", [4 * 2 * H, 1], F32, kind="ExternalInput")
    bc_in = nc.dram_tensor("bc_in", [4 * H, 1], F32, kind="ExternalInput")
    wout_in = nc.dram_tensor("wout_in", [H, 1], F32, kind="ExternalInput")
    bout_in = nc.dram_tensor("bout_in", [1, 1], F32, kind="ExternalInput")
    ident_in = nc.dram_tensor("ident_in", [128, 128], F32, kind="ExternalInput")
    out_t = nc.dram_tensor("out", [1, BN], F32, kind="ExternalOutput")

    with tile.TileContext(nc) as tc:
        with (
            tc.tile_pool(name="persist", bufs=1) as persist,
            tc.tile_pool(name="acolp", bufs=2) as acolp,
            tc.tile_pool(name="lhstp", bufs=2) as lhstp,
            tc.tile_pool(name="hbp", bufs=HB_BUFS) as hbp,
            tc.tile_pool(name="rhbp", bufs=RHB_BUFS) as rhbp,
            tc.tile_pool(name="statep", bufs=2) as statep,
            tc.tile_pool(name="hstp", bufs=5) as hstp,
            tc.tile_pool(name="smallp", bufs=2) as smallp,
            tc.tile_pool(name="pdiff", bufs=4, space="PSUM") as pdiff,
            tc.tile_pool(name="pproj", bufs=2, space="PSUM") as pproj,
            tc.tile_pool(name="ptr", bufs=1, space="PSUM") as ptr,
            tc.tile_pool(name="dml", bufs=3, space="DRAM") as dml,
            tc.tile_pool(name="dms", bufs=2, space="DRAM") as dms,
        ):
            uid = [0]

            def nm(pfx):
                uid[0] += 1
                return f"{pfx}{uid[0]}"

            dma_engines = [nc.sync, nc.scalar, nc.gpsimd]

            def dma_eng(i):
                return dma_engines[i % 3]

            # ---- persistent SBUF ----
            ident = persist.tile([128, 128], F32, name="ident")
            nc.sync.dma_start(ident, ident_in.ap())
            ident_bf = persist.tile([128, 128], BF16, name="ident_bf")
            nc.vector.tensor_copy(ident_bf, ident)
            # operator pairs: rp1 = [A | A2], rp2 = [AT | AT2], per kt
            rp1 = persist.tile([128, KT, 512], F32R, name="rp1")
            rp2 = persist.tile([128, KT, 512], F32R, name="rp2")
            nc.sync.dma_start(
                rp1[:, :, 0:NS],
                r_a.ap().bitcast(F32R).rearrange("(kt p) n -> p kt n", p=128),
            )
            nc.scalar.dma_start(
                rp2[:, :, 0:NS],
                r_at.ap().bitcast(F32R).rearrange("(kt p) n -> p kt n", p=128),
            )
            # weights split into x-part / h-part tiles (base partition 0 each)
            wgx, wgh, wcx, wch, bgr_sb, bgu_sb, bc_sb = {}, {}, {}, {}, {}, {}, {}
            for pi, key in enumerate(PHASES):
                src_g = wg_in.ap()[pi, :, :].bitcast(F32R).rearrange(
                    "(o p) u -> p o u", p=128)
                wgx[key] = persist.tile([H, 5, 2 * H], F32R, name=f"wgx{pi}")
                nc.sync.dma_start(wgx[key], src_g[0:H])
                wgh[key] = persist.tile([H, 5, 2 * H], F32R, name=f"wgh{pi}")
                nc.sync.dma_start(wgh[key], src_g[H:2 * H])
                src_c = wc_in.ap()[pi, :, :].bitcast(F32R).rearrange(
                    "(o p) u -> p o u", p=128)
                wcx[key] = persist.tile([H, 5, H], F32R, name=f"wcx{pi}")
                nc.sync.dma_start(wcx[key], src_c[0:H])
                wch[key] = persist.tile([H, 5, H], F32R, name=f"wch{pi}")
                nc.sync.dma_start(wch[key], src_c[H:2 * H])
                bgr_sb[key] = persist.tile([H, 1], F32, name=f"bgr{pi}")
                nc.sync.dma_start(bgr_sb[key], bg_in.ap()[pi * 128: pi * 128 + H, :])
                bgu_sb[key] = persist.tile([H, 1], F32, name=f"bgu{pi}")
                nc.sync.dma_start(bgu_sb[key], bg_in.ap()[pi * 128 + H: pi * 128 + 2 * H, :])
                bc_sb[key] = persist.tile([H, 1], F32, name=f"bc{pi}")
                nc.sync.dma_start(bc_sb[key], bc_in.ap()[pi * H: (pi + 1) * H, :])
            wout_sb = persist.tile([H, 1], F32, name="wout_sb")
            nc.sync.dma_start(wout_sb, wout_in.ap())
            bout_sb = persist.tile([1, 1], F32, name="bout_sb")
            nc.sync.dma_start(bout_sb, bout_in.ap())

            a2a_in = dml.tile([N, NS], BF16, name="a2a_in", tag="a2a")
            # ---- setup: R_A2 = A^T @ R_A  (lhsT = A streamed col-block-wise) ----
            for mt in range(KT):
                for half in range(2):
                    acol = acolp.tile([128, KT // 2, 128], F32R, name=nm("acol"),
                                      tag="acol")
                    dma_eng(mt * 2 + half).dma_start(
                        acol,
                        a_full.ap()[half * (N // 2):(half + 1) * (N // 2),
                                    mt * 128:(mt + 1) * 128]
                        .bitcast(F32R)
                        .rearrange("(kt p) m -> p kt m", p=128),
                    )
                    if half == 0:
                        pa2 = pdiff.tile([128, NS], F32, name=nm("pa2"), tag="pdiff")
                    for k2 in range(KT // 2):
                        kt = half * (KT // 2) + k2
                        nc.tensor.matmul(
                            pa2, acol[:, k2, :], rp1[:, kt, 0:NS],
                            start=(kt == 0), stop=(kt == KT - 1),
                        )
                nc.vector.tensor_copy(rp1[:, mt, NS:512], pa2)
                a2a_bf = smallp.tile([128, NS], BF16, name=nm("a2abf"), tag="a2abf")
                nc.vector.tensor_copy(a2a_bf, pa2)
                dma_eng(mt).dma_start(a2a_in[mt * 128:(mt + 1) * 128, :], a2a_bf)

            # ---- setup: R_AT2 = A^2[:, sh] via AllToAll of R_A2 + transposes ----
            a2a_out = dml.tile([N, NS], BF16, name="a2a_out", tag="a2a")
            nc.gpsimd.collective_compute(
                "AllToAll", mybir.AluOpType.bypass, replica_groups=RG,
                ins=[a2a_in.opt()], outs=[a2a_out.opt()],
            )
            for c in range(W):
                tin = acolp.tile([128, 2, NS], BF16, name=nm("tin"), tag="acol2")
                dma_eng(c).dma_start(
                    tin,
                    a2a_out[c * NS:(c + 1) * NS, :].rearrange("(h p) n -> p h n", p=128),
                )
                for i2 in range(2):          # which kt within block c
                    kt = 2 * c + i2
                    for h2 in range(2):      # which n-half
                        pt = ptr.tile([128, 128], BF16, name=nm("pt"), tag="ptrb")
                        nc.tensor.transpose(
                            pt, tin[:, h2, i2 * 128:(i2 + 1) * 128],
                            ident_bf[0:128, 0:128],
                        )
                        nc.vector.tensor_copy(
                            rp2[:, kt, NS + h2 * 128: NS + (h2 + 1) * 128], pt
                        )

            # =================================================================
            # helpers
            # =================================================================
            bundles = {}   # name -> (tile, alloc_idx, tag)
            alloc_count = {"hb": 0, "rhb": 0}
            state = {}     # name -> state tile [64, BN]

            def bundle_alloc(name, pool, tag):
                t = pool.tile([H, B, 5, NS], F32R, name=nm("bun_" + name), tag=tag)
                alloc_count[tag] += 1
                bundles[name] = (t, alloc_count[tag], tag)
                return t

            def bundle_get(name):
                t, idx, tag = bundles[name]
                bufs = {"hb": HB_BUFS, "rhb": RHB_BUFS}[tag]
                assert idx > alloc_count[tag] - bufs, \
                    f"bundle {name} slot recycled ({idx} vs {alloc_count[tag]})"
                return t

            def emit_diffusion(src_dram, names, pool_tags):
                """src_dram: node-major [N, 128*len(names)] DRAM AP."""
                Cm = len(names)
                buns, ps = [], []
                for ti, name in enumerate(names):
                    if name in bundles:
                        buns.append(bundle_get(name))
                    else:
                        pool, tag = pool_tags[ti]
                        buns.append(bundle_alloc(name, pool, tag))
                    p1 = pdiff.tile([128, 512], F32, name=nm("p1"), tag="pdiff")
                    p2 = pdiff.tile([128, 512], F32, name=nm("p2"), tag="pdiff")
                    ps.append((p1, p2))
                KC = 4  # kt tiles per readback chunk
                for ck in range(KT // KC):
                    lt = lhstp.tile([128, KC, Cm * 128], F32R, name=nm("lt"), tag="lt")
                    dma_eng(ck).dma_start(
                        lt,
                        src_dram[ck * KC * 128:(ck + 1) * KC * 128, :]
                        .bitcast(F32R).rearrange("(k p) c -> p k c", p=128),
                    )
                    for k2 in range(KC):
                        kt = ck * KC + k2
                        for ti in range(Cm):
                            p1, p2 = ps[ti]
                            lts = lt[:, k2, ti * 128:(ti + 1) * 128]
                            nc.tensor.matmul(p1, lts, rp1[:, kt, :],
                                             start=(kt == 0), stop=(kt == KT - 1))
                            nc.tensor.matmul(p2, lts, rp2[:, kt, :],
                                             start=(kt == 0), stop=(kt == KT - 1))
                for ti in range(Cm):
                    p1, p2 = ps[ti]
                    bun = buns[ti]
                    for b in range(B):
                        # ops (A, A2) -> [:, b, 1:3, :]; (AT, AT2) -> [:, b, 3:5, :]
                        nc.vector.tensor_copy(bun[:, b, 1:3, :], p1[b * H:(b + 1) * H, :])
                        nc.vector.tensor_copy(bun[:, b, 3:5, :], p2[b * H:(b + 1) * H, :])
                return buns

            def set_identity_slot(bun, src_state):
                nc.vector.tensor_copy(
                    bun[:, :, 0, :],
                    src_state.bitcast(F32R).rearrange("p (b n) -> p b n", b=B),
                )

            def emit_allgather(tensors):
                """tensors: state tiles [64, BN] feature-major. Returns gathered
                node-major DRAM tile [N, 128*len(tensors)]."""
                Cg = 128 * len(tensors)
                stg = statep.tile([128, 2, Cg], F32, name=nm("stg"), tag="stg")
                for ti, t in enumerate(tensors):
                    for b in range(B):
                        for nh in range(2):
                            pt = ptr.tile([128, H], F32, name=nm("agt"), tag="ptr")
                            nc.tensor.transpose(
                                pt,
                                t[:, b * NS + nh * 128: b * NS + (nh + 1) * 128],
                                ident[0:H, 0:H],
                            )
                            nc.vector.tensor_copy(
                                stg[:, nh, ti * 128 + b * H: ti * 128 + (b + 1) * H],
                                pt,
                            )
                ag_in = dml.tile([NS, Cg], F32, name=nm("ag_in"), tag="agin")
                nc.sync.dma_start(ag_in.rearrange("(nh p) c -> p nh c", p=128), stg)
                ag_out = dms.tile([N, Cg], F32, name=nm("ag_out"), tag="agout",
                                  addr_space="Shared")
                nc.gpsimd.collective_compute(
                    "AllGather", mybir.AluOpType.bypass, replica_groups=RG,
                    ins=[ag_in.opt()], outs=[ag_out.opt()],
                )
                return ag_out

            def emit_projection(psum_out, wx, wh, parts, out_dim):
                """psum_out [out_dim, BN]; parts: (bundle_name, 'x'|'h')."""
                mms = []
                for (bname, pos) in parts:
                    if bname is None:
                        continue
                    bun = bundle_get(bname)
                    wt = wx if pos == "x" else wh
                    for op in range(5):
                        mms.append((wt[:, op, :], bun[:, :, op, :]))
                assert mms
                pv = psum_out.rearrange("p (b n) -> p b n", b=B)
                for i, (wap, rap) in enumerate(mms):
                    nc.tensor.matmul(pv, wap, rap,
                                     start=(i == 0), stop=(i == len(mms) - 1))

            def emit_cell(ph, l, x_name, h_name, cid):
                key = (ph, l)
                hs_t = state[h_name] if h_name is not None else None
                gps = pproj.tile([2 * H, BN], F32, name=nm("gps"), tag="pproj")
                emit_projection(gps, wgx[key], wgh[key],
                                [(x_name, "x"), (h_name, "h")], 2 * H)
                r_t = statep.tile([H, BN], F32, name=nm("r"), tag="r")
                nc.scalar.activation(r_t, gps[0:H, :], AF.Sigmoid, bias=bgr_sb[key])
                u_t = statep.tile([H, BN], F32, name=nm("u"), tag="u")
                nc.scalar.activation(u_t, gps[H:2 * H, :], AF.Sigmoid, bias=bgu_sb[key])

                rh_name = None
                if h_name is not None:
                    rh_t = statep.tile([H, BN], F32, name=nm("rh"), tag="rh")
                    nc.vector.tensor_mul(rh_t, r_t, hs_t)
                    rh_name = f"rh_{cid}"
                    ag = emit_allgather([rh_t])
                    buns = emit_diffusion(ag[:, :], [rh_name], [(rhbp, "rhb")])
                    set_identity_slot(buns[0], rh_t)

                cps = pproj.tile([H, BN], F32, name=nm("cps"), tag="pproj")
                emit_projection(cps, wcx[key], wch[key],
                                [(x_name, "x"), (rh_name, "h")], H)
                cand_t = statep.tile([H, BN], F32, name=nm("cand"), tag="cand")
                nc.scalar.activation(cand_t, cps, AF.Tanh, bias=bc_sb[key])

                hn = hstp.tile([H, BN], F32, name=nm("h"), tag="hst")
                tmp = statep.tile([H, BN], F32, name=nm("tmp"), tag="tmp")
                if h_name is None:
                    nc.vector.tensor_mul(tmp, u_t, cand_t)
                    nc.vector.tensor_sub(hn, cand_t, tmp)        # (1-u)*c
                else:
                    tmp2 = statep.tile([H, BN], F32, name=nm("tmp2"), tag="tmp2")
                    nc.vector.tensor_sub(tmp, hs_t, cand_t)
                    nc.vector.tensor_mul(tmp2, u_t, tmp)
                    nc.vector.tensor_add(hn, cand_t, tmp2)       # c + u*(h-c)
                sname = f"h{l}_{cid}"
                state[sname] = hn
                return sname

            def gather_and_diffuse(names):
                ag = emit_allgather([state[nm_] for nm_ in names])
                buns = emit_diffusion(ag[:, :], names, [(hbp, "hb")] * len(names))
                for bun, nm_ in zip(buns, names):
                    set_identity_slot(bun, state[nm_])

            # =================================================================
            # XP: identity slots + diffusion of all 3 timesteps (global, no AG)
            # =================================================================
            for t in range(SEQ):
                bun = bundle_alloc(f"xp_t{t}", hbp, "hb")
                nc.sync.dma_start(
                    bun[:, :, 0, :],
                    xp_fm.ap()[:, t * BN:(t + 1) * BN]
                    .bitcast(F32R).rearrange("p (b n) -> p b n", b=B),
                )
            emit_diffusion(xp_nm.ap()[:, 0:256], ["xp_t0", "xp_t1"], None)
            emit_diffusion(xp_nm.ap()[:, 256:384], ["xp_t2"], None)

            # =================================================================
            # cells
            # =================================================================
            emit_cell("enc", 0, "xp_t0", None, "e0l0")
            gather_and_diffuse(["h0_e0l0"])
            emit_cell("enc", 1, "h0_e0l0", None, "e0l1")

            h0_prev, h1_prev = "h0_e0l0", "h1_e0l1"
            for t in (1, 2):
                s0 = emit_cell("enc", 0, f"xp_t{t}", h0_prev, f"e{t}l0")
                gather_and_diffuse([s0, h1_prev])
                s1 = emit_cell("enc", 1, s0, h1_prev, f"e{t}l1")
                h0_prev, h1_prev = s0, s1

            d0 = emit_cell("dec", 0, None, h0_prev, "d0l0")
            gather_and_diffuse([d0, h1_prev])
            d1 = emit_cell("dec", 1, d0, h1_prev, "d0l1")

            # output projection: o = wout.T @ h1_dec + bout -> [1, BN]
            ops = pproj.tile([1, BN], F32, name="ops", tag="pproj")
            nc.tensor.matmul(ops, wout_sb, state[d1], start=True, stop=True)
            out_sb = smallp.tile([1, BN], F32, name="out_sb", tag="outsb")
            nc.vector.tensor_scalar_add(out_sb, ops, bout_sb)
            nc.sync.dma_start(out_t.ap(), out_sb)

    nc.compile()
    return nc


def make_in_maps(inputs):
    adj = np.asarray(inputs["adj"], np.float64)
    A = adj + np.eye(N) * 1e-6
    A = (A / (A.sum(axis=1, keepdims=True) + 1e-8)).astype(np.float32)
    AT = np.ascontiguousarray(A.T)
    xp = (np.asarray(inputs["inputs"], np.float32)[..., None]
          @ np.asarray(inputs["in_proj_w"], np.float32)
          + np.asarray(inputs["in_proj_b"], np.float32))  # (B, SEQ, N, H)
    xp_nm = np.ascontiguousarray(xp.transpose(2, 1, 0, 3).reshape(N, SEQ * B * H))
    wg = np.ascontiguousarray(np.concatenate(
        [np.asarray(inputs["enc_gate_w"], np.float32),
         np.asarray(inputs["dec_gate_w"], np.float32)], axis=0))
    wc = np.ascontiguousarray(np.concatenate(
        [np.asarray(inputs["enc_cand_w"], np.float32),
         np.asarray(inputs["dec_cand_w"], np.float32)], axis=0))
    bg = np.ascontiguousarray(np.concatenate(
        [np.asarray(inputs["enc_gate_b"], np.float32),
         np.asarray(inputs["dec_gate_b"], np.float32)], axis=0).reshape(4 * 2 * H, 1))
    bc = np.ascontiguousarray(np.concatenate(
        [np.asarray(inputs["enc_cand_b"], np.float32),
         np.asarray(inputs["dec_cand_b"], np.float32)], axis=0).reshape(4 * H, 1))
    wout = np.ascontiguousarray(np.asarray(inputs["out_proj_w"], np.float32))
    bout = np.asarray(inputs["out_proj_b"], np.float32).reshape(1, 1)
    ident = np.eye(128, dtype=np.float32)

    in_maps = []
    for r in range(W):
        sh = slice(r * NS, (r + 1) * NS)
        xp_fm = np.ascontiguousarray(
            xp[:, :, sh, :].transpose(3, 1, 0, 2).reshape(H, SEQ * B * NS))
        in_maps.append({
            "r_a": np.ascontiguousarray(AT[:, sh]),
            "r_at": np.ascontiguousarray(A[:, sh]),
            "a_full": A,
            "xp_nm": xp_nm,
            "xp_fm": xp_fm,
            "wg_in": wg, "wc_in": wc, "bg_in": bg, "bc_in": bc,
            "wout_in": wout, "bout_in": bout, "ident_in": ident,
        })
    return in_maps


def assemble_output(results):
    out = np.zeros((B, 1, N), np.float32)
    for r in range(W):
        res = results[r]["out"]  # [1, BN]
        for b in range(B):
            out[b, 0, r * NS:(r + 1) * NS] = res[0, b * NS:(b + 1) * NS]
    return out


_CACHE = {}


def get_program():
    if "nc" not in _CACHE:
        _CACHE["nc"] = build_program()
    return _CACHE["nc"]


def kernel(**inputs):
    nc = get_program()
    in_maps = make_in_maps(inputs)
    res = run_bass_kernel_spmd(nc, in_maps, core_ids=list(range(W)))
    return assemble_output(res.results)
